# revision 4
# baseline (speedup 1.0000x reference)
"""Mass-spring substep integrator on 8 Trainium2 NeuronCores — v2.

Layout/dataflow redesign of the baseline:
  - node-major SBUF state: pos_sb[p, k, m] with m = b*3+c (12 floats per node
    record, contiguous) -> all DRAM publishes are single contiguous DMAs
    instead of 24 strided 4B-element copies.
  - the force AllReduce is replaced by ReduceScatter (each core integrates
    only its 1/8 node chunk, held in uniform own_pos/own_vel tiles so the
    SPMD program has no core-dependent addressing) followed by an AllGather
    of the updated positions whose output IS the gather table (agout).
  - trajectory outputs are per-core own-chunk slices, assembled on the host.
  - the partner-position gather stays one indirect DMA per grid column
    (the only per-partition-indexed instrument this toolchain supports).

Grid plan (host): balanced 8-way edge shards; nodes ranked by max per-shard
degree, dealt round-robin onto 128 partitions x 784 ranks; shared
degree-profile template across shards/partitions so owner-broadcast and
segmented reduction are uniform strided vector ops.
"""

import numpy as np

import concourse.bass as bass
import concourse.mybir as mybir
import concourse.tile as tile
from concourse.bass_utils import run_bass_kernel_spmd

# Problem constants (must match the reference)
B, NV, NE, SUBSTEPS = 4, 100000, 400000, 10
DT = 0.01
K_SPRING = 1000.0
MASS = 1.0
DAMP = 0.999
ACT_SCALE = 0.1
EPS = 1e-6
GRAVITY_Y = -9.8

P = 128            # SBUF partitions
NSHARD = 8         # edge shards == cores
NVP = 784          # node ranks per partition (8 chunks x 98)
NVTOT = NVP * P    # padded node count (100352)
NCH = 8            # position chunks (== cores)
KCH = NVP // NCH   # ranks per chunk (98)
M = 3 * B          # floats per node record (12)
CHM = KCH * M      # own-chunk floats per partition (1176)


# ---------------------------------------------------------------------------
# walrus workaround: this toolchain accepts only ONE sync-wait per
# instruction; split extra waits onto fresh same-engine NOPs.
# ---------------------------------------------------------------------------
_ctr = [0]


def _split_multi_waits(nc):
    for f in nc.m.functions:
        for b in f.blocks:
            old = b.instructions
            new = []
            changed = False
            for inst in old:
                si = inst.sync_info
                if si is not None and si.on_wait is not None and len(si.on_wait) > 1:
                    waits = list(si.on_wait)
                    for w in waits[:-1]:
                        _ctr[0] += 1
                        nop = mybir.InstNoOp(
                            name=f"SPLITW-{_ctr[0]}",
                            engine=inst.engine,
                            ins=[], outs=[],
                            sync_info=mybir.SyncInfo(on_wait=[w], on_update=[]),
                        )
                        new.append(nop)
                    si.on_wait = waits[-1:]
                    changed = True
                new.append(inst)
            if changed:
                b.instructions = new


class _TileContext(tile.TileContext):
    def __exit__(self, *args):
        r = super().__exit__(*args)
        if args[0] is None:
            _split_multi_waits(self.nc)
        return r


# ---------------------------------------------------------------------------
# Host-side plan construction (static, depends only on the edge list)
# ---------------------------------------------------------------------------
class Plan:
    pass


def build_plan(edges, nv, ne):
    """Balanced 8-shard split, node ranking, shared degree template and
    per-shard slot tables (agout-row partner indices)."""
    rng = np.random.RandomState(0)

    i_idx = edges[:, 0].astype(np.int64)
    j_idx = edges[:, 1].astype(np.int64)

    # --- balanced split of edges into NSHARD shards (per-node incidence) ---
    order = rng.permutation(ne)
    cnt = np.zeros((NSHARD, nv), np.int32)
    shard_of_edge = np.zeros(ne, np.int8)
    ii, jj = i_idx[order], j_idx[order]
    for t in range(ne):
        u = ii[t]
        v = jj[t]
        s = int(np.argmin(cnt[:, u] + cnt[:, v]))
        shard_of_edge[order[t]] = s
        cnt[s, u] += 1
        cnt[s, v] += 1

    # --- refinement: push per-node max shard-degree toward ceil(deg/8) ---
    deg_tot = np.zeros(nv, np.int64)
    np.add.at(deg_tot, i_idx, 1)
    np.add.at(deg_tot, j_idx, 1)
    cap_u = -(-deg_tot // NSHARD)
    # incidence CSR: for node u, list of (edge id) sorted by u
    inc_u = np.concatenate([i_idx, j_idx])
    inc_e = np.concatenate([np.arange(ne), np.arange(ne)])
    so = np.argsort(inc_u, kind="stable")
    inc_u, inc_e = inc_u[so], inc_e[so]
    inc_off = np.zeros(nv + 1, np.int64)
    inc_off[1:] = np.cumsum(np.bincount(inc_u, minlength=nv))
    for _ in range(8):
        maxd = cnt.max(axis=0)
        bad = np.where(maxd > cap_u)[0]
        if len(bad) == 0:
            break
        moved = 0
        for u in bad:
            h_max = int(np.argmax(cnt[:, u]))
            if cnt[h_max, u] <= cap_u[u]:
                continue
            for t in range(inc_off[u], inc_off[u + 1]):
                e = inc_e[t]
                if shard_of_edge[e] != h_max:
                    continue
                a, b2 = int(i_idx[e]), int(j_idx[e])
                v = b2 if a == u else a
                # move e to the shard minimizing the worse of the two ends
                score = np.maximum(cnt[:, u] - cap_u[u], cnt[:, v] - cap_u[v])
                score[h_max] = 10**9
                h_new = int(np.argmin(score))
                if (max(cnt[h_new, u] + 1 - cap_u[u],
                        cnt[h_new, v] + 1 - cap_u[v])
                        < max(cnt[h_max, u] - cap_u[u],
                              cnt[h_max, v] - cap_u[v])):
                    shard_of_edge[e] = h_new
                    cnt[h_max, u] -= 1
                    cnt[h_max, v] -= 1
                    cnt[h_new, u] += 1
                    cnt[h_new, v] += 1
                    moved += 1
                    if cnt[h_max, u] <= cap_u[u]:
                        break
        if moved == 0:
            break

    deg_h = cnt  # [NSHARD, NV]

    # --- node ranking: sort by max shard-degree desc, deal round-robin ---
    key = deg_h.max(axis=0)
    node_order = np.argsort(-key, kind="stable")
    node_order_pad = np.concatenate([node_order, np.arange(nv, NVTOT)])
    grid_nodes = node_order_pad.reshape(NVP, P)  # [k, p]

    # --- shared degree template: dk[k] = max over shards & partitions ---
    degh_pad = np.zeros((NSHARD, NVTOT), np.int32)
    degh_pad[:, :nv] = deg_h
    dk = np.max(degh_pad[:, grid_nodes], axis=(0, 2))  # [NVP]
    rk_order = np.argsort(-dk, kind="stable")
    grid_nodes = grid_nodes[rk_order]
    dk = dk[rk_order]
    p_of = np.zeros(NVTOT, np.int32)
    k_of = np.zeros(NVTOT, np.int32)
    p_of[grid_nodes.ravel()] = np.tile(np.arange(P), NVP)
    k_of[grid_nodes.ravel()] = np.repeat(np.arange(NVP), P)

    seg_start = np.zeros(NVP + 1, np.int64)
    seg_start[1:] = np.cumsum(dk)
    J = int(seg_start[-1])

    # degree classes: runs of equal dk with dk >= 1
    classes = []
    k = 0
    while k < NVP:
        d = int(dk[k])
        k2 = k
        while k2 < NVP and dk[k2] == d:
            k2 += 1
        if d >= 1:
            classes.append((k, k2, d))
        k = k2

    # split classes into chunks of bounded slot count (for SBUF);
    # uneven split: small final chunk shortens the pre-ReduceScatter tail
    fracs = [0.45, 0.45, 0.10]
    targets = [int(J * f) + 1 for f in fracs]
    cls_chunks = [[]]
    cur = 0
    ti = 0
    for (ka, kb, d) in classes:
        k0 = ka
        while k0 < kb:
            room = max((targets[min(ti, len(targets) - 1)] - cur) // d, 0)
            take = min(kb - k0, room)
            if take == 0:
                cls_chunks.append([])
                cur = 0
                ti += 1
                continue
            cls_chunks[-1].append((k0, k0 + take, d))
            cur += take * d
            k0 += take
    cls_chunks = [ch for ch in cls_chunks if ch]
    chunk_bounds = [
        (int(seg_start[ch[0][0]]), int(seg_start[ch[-1][1]]))
        for ch in cls_chunks
    ]

    # --- per-shard slot tables; partner index = agout-flat row ---
    # agout layout [chunk r, partition p, local rank kl] ->
    # row(v) = (k//KCH)*(P*KCH) + p*KCH + k%KCH
    agrow = ((k_of // KCH).astype(np.int64) * (P * KCH)
             + p_of.astype(np.int64) * KCH + (k_of % KCH))
    part_idx = np.zeros((NSHARD, P, J), np.int32)
    eidx_slot = np.full((NSHARD, P, J), -1, np.int64)

    for h in range(NSHARD):
        sel = shard_of_edge == h
        eu = np.concatenate([i_idx[sel], j_idx[sel]])
        ev = np.concatenate([j_idx[sel], i_idx[sel]])
        ee = np.concatenate([np.nonzero(sel)[0]] * 2)
        owner_p = p_of[eu]
        owner_k = k_of[eu]
        so = np.lexsort((ee, owner_k, owner_p))
        eu, ev, ee = eu[so], ev[so], ee[so]
        owner_p, owner_k = owner_p[so], owner_k[so]
        grp = owner_p.astype(np.int64) * NVP + owner_k
        uniq, first = np.unique(grp, return_index=True)
        within = np.arange(len(grp)) - np.repeat(
            first, np.diff(np.append(first, len(grp))))
        jpos = seg_start[owner_k] + within
        part_idx[h, owner_p, jpos] = agrow[ev]
        eidx_slot[h, owner_p, jpos] = ee
        # pad slots: any valid row (0); force zeroed via mask

    plan = Plan()
    plan.nv, plan.ne, plan.J = nv, ne, J
    plan.classes = classes
    plan.cls_chunks = cls_chunks
    plan.chunk_bounds = chunk_bounds
    plan.seg_start = seg_start
    plan.part_idx = part_idx
    plan.eidx_slot = eidx_slot
    plan.p_of, plan.k_of = p_of, k_of
    plan.grid_nodes = grid_nodes
    plan.dk = dk
    return plan


def host_state_inputs(plan, input_pos, input_vel):
    """pos0 full [P, NVP*M] node-major; per-chunk own pos/vel [NCH][P, CHM]."""
    nv = plan.nv
    nb = input_pos.shape[0]
    gn = plan.grid_nodes  # [k, p]
    valid = gn < nv
    gp = np.clip(gn, 0, nv - 1)
    ps = input_pos[:, gp].copy()   # [b, k, p, 3]
    vs = input_vel[:, gp].copy()
    ps[:, ~valid] = 0.0
    vs[:, ~valid] = 0.0
    # node-major: pos0[p, k*M + b*3 + c]
    pos = ps.transpose(2, 1, 0, 3).reshape(P, NVP * M)
    vel = vs.transpose(2, 1, 0, 3).reshape(P, NVP * M)
    pos = np.ascontiguousarray(pos, dtype=np.float32)
    vel = np.ascontiguousarray(vel, dtype=np.float32)
    pos_own = [np.ascontiguousarray(pos[:, h * CHM:(h + 1) * CHM])
               for h in range(NCH)]
    vel_own = [np.ascontiguousarray(vel[:, h * CHM:(h + 1) * CHM])
               for h in range(NCH)]
    return np.ascontiguousarray(pos.astype(np.float16)), pos_own, vel_own


def host_shard_inputs(plan, h, input_action, rest_len):
    """Per-core tables: pidx [P,J] i32, rest [P,J] f32, act [P,J*B],
    mask [P,J] (1 real, 0 pad)."""
    J = plan.J
    nb = input_action.shape[0]
    e = plan.eidx_slot[h]
    pad = e < 0
    ec = np.clip(e, 0, plan.ne - 1)
    rest = rest_len[ec].astype(np.float32)
    rest[pad] = 1.0
    act = input_action[:, ec].astype(np.float32)  # [b, P, J]
    act[:, pad] = 0.0
    act = np.ascontiguousarray(act.transpose(1, 2, 0).reshape(P, J * nb))
    mask = np.ascontiguousarray((~pad).astype(np.float32))
    return {
        "pidx": np.ascontiguousarray(plan.part_idx[h]),
        "rest_s": np.ascontiguousarray(rest),
        "act_s": act,
        "mask_s": mask,
    }


def assemble_output(plan, chunks_traj, nb):
    """chunks_traj: list of 8 arrays [S+1, P, CHM] (core h -> chunk h).
    Returns [nb, S+1, NV, 3] in original node order."""
    S1 = chunks_traj[0].shape[0]
    full = np.empty((S1, P, NVP * M), np.float32)
    for h in range(NCH):
        full[:, :, h * CHM:(h + 1) * CHM] = chunks_traj[h]
    # full[s, p, k*M + b*3 + c] -> [s, p, k, b, c]
    t = full.reshape(S1, P, NVP, nb, 3)
    pv = plan.p_of[: plan.nv]
    kv = plan.k_of[: plan.nv]
    out = t[:, pv, kv]              # [S1, NV, nb, 3]
    return np.ascontiguousarray(out.transpose(2, 0, 1, 3))


# ---------------------------------------------------------------------------
# Device kernel
# ---------------------------------------------------------------------------
def build_bass(plan, substeps, nb):
    J = plan.J
    NPM = NVP * M
    f32 = mybir.dt.float32

    nc = bass.Bass(num_devices=8)
    f16 = mybir.dt.float16
    pos0 = nc.dram_tensor("pos0", [P, NPM], f16, kind="ExternalInput")
    pos0_own = nc.dram_tensor("pos0_own", [P, CHM], f32, kind="ExternalInput")
    vel0_own = nc.dram_tensor("vel0_own", [P, CHM], f32, kind="ExternalInput")
    pidx = nc.dram_tensor("pidx", [P, J], mybir.dt.int32, kind="ExternalInput")
    rest_in = nc.dram_tensor("rest_s", [P, J], f32, kind="ExternalInput")
    act_in = nc.dram_tensor("act_s", [P, J * nb], f32, kind="ExternalInput")
    mask_in = nc.dram_tensor("mask_s", [P, J], f32, kind="ExternalInput")

    opos = nc.dram_tensor("opos", [substeps + 1, P, CHM], f32,
                          kind="ExternalOutput")
    ovel = nc.dram_tensor("ovel", [substeps + 1, P, CHM], f32,
                          kind="ExternalOutput")

    cc_in = nc.dram_tensor("cc_in", [NCH, P, CHM], f32, kind="Internal")
    rs_out = nc.dram_tensor("rs_out", [P, CHM], f32, kind="Internal")
    ag_in = nc.dram_tensor("ag_in", [P, CHM], f16, kind="Internal")
    agout = nc.dram_tensor("agout", [NCH, P, CHM], f16, kind="Internal")

    chunks = plan.chunk_bounds
    maxchunk = max(hi - lo for (lo, hi) in chunks)

    with _TileContext(nc) as tc:
        with tc.tile_pool(name="state", bufs=1) as pool:
            pos = pool.tile([P, NPM], f16, name="pos")          # full, global k (fp16)
            fsum = pool.tile([P, NPM], f32, name="fsum")
            own_pos = pool.tile([P, CHM], f32, name="own_pos")
            own_vel = pool.tile([P, CHM], f32, name="own_vel")
            own_f = pool.tile([P, CHM], f32, name="own_f")
            pidx_sb = pool.tile([P, J], mybir.dt.int32, name="pidx_sb")
            kr = pool.tile([P, J * nb], f32, name="kr")
            mk = pool.tile([P, J], f32, name="mk")
            s2 = pool.tile([P, maxchunk * nb], f32, name="s2")
            sq = pool.tile([P, maxchunk * nb], f32, name="sq")
            rem = pool.tile([P, 2 * maxchunk * M], f16, name="rem")
            own_pos16 = pool.tile([P, CHM], f16, name="own_pos16")
            eps_t = pool.tile([P, 1], f32, name="eps_t")

            pos_mk = pos[:].rearrange("p (k m) -> p k m", m=M)
            fsum_mk = fsum[:].rearrange("p (k m) -> p k m", m=M)
            ag_flat = agout[:].rearrange("r p (kl m) -> (r p kl) m", m=M)
            ag_prc = agout[:].rearrange("r p c -> p r c")
            cc_prc = cc_in[:].rearrange("r p c -> p r c")

            def _ins_bcast(ap, pos_idx, count):
                dims = [list(x) for x in ap.ap]
                dims.insert(pos_idx, [0, count])
                return bass.AP(ap.tensor, ap.offset, dims)

            # ---- one-time setup ----
            nc.vector.memset(eps_t[:], float(EPS))
            nc.vector.memset(fsum[:], 0.0)   # covers dk=0 ranks forever
            nc.sync.dma_start(pos[:], pos0[:])
            nc.sync.dma_start(own_pos[:], pos0_own[:])
            nc.sync.dma_start(own_vel[:], vel0_own[:])
            nc.sync.dma_start(pidx_sb[:], pidx[:])
            nc.sync.dma_start(mk[:], mask_in[:])
            # kr[p, j, b] = K * rest[p, j] * (1 + ACT_SCALE * tanh(act))
            nc.sync.dma_start(kr[:], act_in[:])
            nc.scalar.activation(kr[:], kr[:],
                                 mybir.ActivationFunctionType.Tanh)
            nc.vector.tensor_scalar(
                out=kr[:], in0=kr[:], scalar1=float(ACT_SCALE),
                scalar2=float(1.0), op0=mybir.AluOpType.mult,
                op1=mybir.AluOpType.add)
            rest_t = sq[:, 0:J] if maxchunk * nb >= J else None
            assert rest_t is not None, (maxchunk * nb, J)
            nc.sync.dma_start(rest_t, rest_in[:])
            kr_v = kr[:].rearrange("p (j b) -> p j b", b=nb)
            rest_b = _ins_bcast(rest_t, 2, nb)
            nc.vector.tensor_tensor(out=kr_v, in0=kr_v, in1=rest_b,
                                    op=mybir.AluOpType.mult)
            nc.vector.tensor_scalar_mul(kr[:], kr[:], float(K_SPRING))
            # fold the pad mask into kr: kr=0 on pad slots; with the
            # (kr*invlen - K)*mask formulation we still need mask; instead
            # multiply kr by mask AND keep mask for the -K term.
            mk_b = _ins_bcast(mk[:], 2, nb)
            nc.vector.tensor_tensor(out=kr_v, in0=kr_v, in1=mk_b,
                                    op=mybir.AluOpType.mult)

            # initial trajectory + initial gather table
            nc.sync.dma_start(opos[0], own_pos[:])
            nc.sync.dma_start(ovel[0], own_vel[:])
            nc.sync.dma_start(
                ag_prc, pos[:].rearrange("p (r c) -> p r c", r=NCH))
            # seed cc_in once: r-chunks beyond the last active rank stay
            # zero forever (fsum is only rewritten on dk>=1 ranks)
            for r in range(NCH):
                nc.sync.dma_start(cc_in[r],
                                  fsum[:, r * CHM:(r + 1) * CHM])
            kb_last = plan.cls_chunks[-1][-1][1]

            # ---- substeps (statically unrolled) ----
            for s in range(substeps):
                written_r = 0
                for ci, (lo, hi) in enumerate(chunks):
                    cw = hi - lo
                    roff = (ci % 2) * maxchunk * M
                    remc = rem[:, roff:roff + cw * M]
                    rem_v = remc.rearrange("p (j r) -> p j r", r=M)
                    # 1) gather partner records, one column per call
                    for j in range(lo, hi):
                        nc.gpsimd.indirect_dma_start(
                            out=rem[:, roff + (j - lo) * M:
                                    roff + (j - lo + 1) * M],
                            out_offset=None,
                            in_=ag_flat,
                            in_offset=bass.IndirectOffsetOnAxis(
                                ap=pidx_sb[:, j:j + 1], axis=0),
                        )
                    # 2) d' = own - rem (per degree class)
                    for (ka, kb, d) in plan.cls_chunks[ci]:
                        s0 = int(plan.seg_start[ka]) - lo
                        nk = kb - ka
                        dst = rem_v[:, s0:s0 + nk * d, :].rearrange(
                            "p (n dd) r -> p n dd r", dd=d)
                        src = pos_mk[:, ka:kb, :].rearrange("p n m -> p n m")
                        src = _ins_bcast(src, 2, d)
                        nc.vector.tensor_tensor(
                            out=dst, in0=src, in1=dst,
                            op=mybir.AluOpType.subtract)
                    # 3) s2[j, b] = sum_c d_c^2
                    s2c = s2[:, 0:cw * nb]
                    s2v = s2c.rearrange("p (j b) -> p j b", b=nb)
                    sqc = sq[:, 0:cw * nb]
                    sqv = sqc.rearrange("p (j b) -> p j b", b=nb)
                    rem_jbc = remc.rearrange(
                        "p (j b c) -> p j b c", b=nb, c=3)
                    cviews = [rem_jbc[:, :, :, c] for c in range(3)]
                    nc.vector.tensor_tensor(out=s2v, in0=cviews[0],
                                            in1=cviews[0],
                                            op=mybir.AluOpType.mult)
                    for c in (1, 2):
                        nc.vector.tensor_tensor(out=sqv, in0=cviews[c],
                                                in1=cviews[c],
                                                op=mybir.AluOpType.mult)
                        nc.vector.tensor_tensor(out=s2v, in0=s2v, in1=sqv,
                                                op=mybir.AluOpType.add)
                    # invlen = 1/sqrt(s2+eps) (Act), t = kr*invlen - K*mask
                    nc.scalar.activation(
                        s2c, s2c, mybir.ActivationFunctionType.Sqrt,
                        bias=eps_t[:])
                    nc.vector.reciprocal(sqc, s2c)
                    nc.vector.tensor_tensor(
                        out=s2c, in0=sqc, in1=kr[:, lo * nb:hi * nb],
                        op=mybir.AluOpType.mult)
                    # s2 = kr*invlen; subtract K*mask -> coef' (pads: 0-0=0)
                    mkc = mk[:, lo:hi]
                    mk_bc = _ins_bcast(mkc, 2, nb)
                    nc.vector.scalar_tensor_tensor(
                        out=s2v, in0=mk_bc, scalar=float(-K_SPRING),
                        in1=s2v, op0=mybir.AluOpType.mult,
                        op1=mybir.AluOpType.add)
                    # 4) f = coef' * d'  (= (K - kr/len)*(rem-own));
                    # coef stays f32 (can reach ~1e6 for near-zero springs,
                    # which would overflow fp16; the product is bounded)
                    coef_b = _ins_bcast(s2v, 3, 3)
                    nc.vector.tensor_tensor(
                        out=rem_jbc, in0=rem_jbc, in1=coef_b,
                        op=mybir.AluOpType.mult)
                    # 5) segmented reduce -> fsum
                    for (ka, kb, d) in plan.cls_chunks[ci]:
                        s0 = int(plan.seg_start[ka]) - lo
                        nk = kb - ka
                        src = rem_v[:, s0:s0 + nk * d, :].rearrange(
                            "p (n dd) r -> p n r dd", dd=d)
                        dst = fsum_mk[:, ka:kb, :]
                        nc.vector.tensor_reduce(
                            out=dst, in_=src, axis=mybir.AxisListType.X,
                            op=mybir.AluOpType.add)
                    # stream completed cc_in r-chunks (overlaps later gathers)
                    kb_ci = plan.cls_chunks[ci][-1][1]
                    while (written_r + 1) * KCH <= kb_ci:
                        nc.sync.dma_start(
                            cc_in[written_r],
                            fsum[:, written_r * CHM:(written_r + 1) * CHM])
                        written_r += 1

                # 6) flush remaining active r-chunks; ReduceScatter
                while written_r * KCH < kb_last:
                    nc.sync.dma_start(
                        cc_in[written_r],
                        fsum[:, written_r * CHM:(written_r + 1) * CHM])
                    written_r += 1
                nc.gpsimd.collective_compute(
                    "ReduceScatter", mybir.AluOpType.add,
                    replica_groups=[list(range(8))],
                    ins=[cc_in[:]], outs=[rs_out[:]],
                )
                nc.sync.dma_start(own_f[:], rs_out[:])
                # 7) integrate own chunk
                nc.vector.scalar_tensor_tensor(
                    out=own_f[:], in0=own_f[:], scalar=float(DT / MASS),
                    in1=own_vel[:], op0=mybir.AluOpType.mult,
                    op1=mybir.AluOpType.add)
                ycols = own_f[:].rearrange(
                    "p (kl b c) -> p kl b c", b=nb, c=3)[:, :, :, 1]
                nc.vector.tensor_scalar_add(ycols, ycols,
                                            float(GRAVITY_Y * DT))
                nc.vector.tensor_scalar_mul(own_vel[:], own_f[:], float(DAMP))
                nc.vector.scalar_tensor_tensor(
                    out=own_pos[:], in0=own_vel[:], scalar=float(DT),
                    in1=own_pos[:], op0=mybir.AluOpType.mult,
                    op1=mybir.AluOpType.add)
                # 8) position exchange first (AG gates the next substep),
                # trajectory writes overlap the AG
                if s + 1 < substeps:
                    nc.vector.tensor_scalar_mul(own_pos16[:], own_pos[:], 1.0)
                    nc.sync.dma_start(ag_in[:], own_pos16[:])
                nc.sync.dma_start(opos[s + 1], own_pos[:])
                nc.sync.dma_start(ovel[s + 1], own_vel[:])
                if s + 1 < substeps:
                    nc.gpsimd.collective_compute(
                        "AllGather", mybir.AluOpType.bypass,
                        replica_groups=[list(range(8))],
                        ins=[ag_in[:]], outs=[agout[:]],
                    )
                    # refresh full owner-side positions (overlaps gathers)
                    nc.sync.dma_start(
                        pos[:].rearrange("p (r c) -> p r c", r=NCH), ag_prc)

    return nc


# ---------------------------------------------------------------------------
# Entry point
# ---------------------------------------------------------------------------
_cache = {}


def _get_plan_and_bass(edges, nv, ne, substeps, nb):
    kh = (hash(edges.tobytes()), nv, ne, substeps, nb)
    if kh not in _cache:
        plan = build_plan(edges, nv, ne)
        nc = build_bass(plan, substeps, nb)
        _cache[kh] = (plan, nc)
    return _cache[kh]


def kernel(input_action, input_pos, input_vel, rest_len, edges):
    input_action = np.asarray(input_action, np.float32)
    input_pos = np.asarray(input_pos, np.float32)
    input_vel = np.asarray(input_vel, np.float32)
    rest_len = np.asarray(rest_len, np.float32)
    edges = np.asarray(edges, np.int32)

    nb, nv, _ = input_pos.shape
    ne = edges.shape[0]
    plan, nc = _get_plan_and_bass(edges, nv, ne, SUBSTEPS, nb)

    pos0, pos_own, vel_own = host_state_inputs(plan, input_pos, input_vel)
    in_maps = []
    for c in range(8):
        im = {"pos0": pos0, "pos0_own": pos_own[c], "vel0_own": vel_own[c]}
        im.update(host_shard_inputs(plan, c, input_action, rest_len))
        in_maps.append(im)
    res = run_bass_kernel_spmd(nc, in_maps, core_ids=list(range(8)))

    pos_chunks = [res.results[c]["opos"] for c in range(8)]
    vel_chunks = [res.results[c]["ovel"] for c in range(8)]
    out_pos = assemble_output(plan, pos_chunks, nb)
    out_vel = assemble_output(plan, vel_chunks, nb)
    return out_pos, out_vel
